# revision 16
# baseline (speedup 1.0000x reference)
"""Gaussian resampling kernel for Trainium2 (8 NeuronCores, SPMD).

Computes, for each batch row b:
    e = cumsum(d); c = e - d/2
    w[t, s] = softmax_s(-(t - c_s)^2 / 10)   (masked s get weight 0)
    out[t, :] = sum_s w[t, s] * x[s, :]

Strategy (fp16 num/den output, host-side normalization):
  - Host precomputes c (float64 cumsum) and folds the mask in by moving
    masked centers to -1e4 (their exp underflows to exactly 0 in fp32).
  - Data-parallel over batch: 2 batches per core on 8 cores, sorted by
    valid length and paired into per-core slots of similar length.
  - Scores in [S, T] layout (tokens on partitions): two ACT passes
    (Square with per-partition bias, then Exp emitting fp16). Banded
    sparsity: each 128-token chunk is only active in a contiguous frame
    range (union over the slot's 8 batches, baked into the program).
  - x is scaled by 256 and a 256-column appended: the matmul produces
    256*numerator (T, D) and 256*denominator (T, 1) together, both in
    fp16 normal range (no subnormal precision loss). The softmax divide
    happens on the HOST after the gather — the device never normalizes,
    so PSUM evacuation is a pure dtype-converting copy.
  - PSUM is organized as PAIR regions [128, 2, 1024] f32 (4 banks): two
    consecutive 128-frame output chunks side by side. ONE copy op per
    pair (strided 3D access pattern, FD=2x769) evacuates num+den
    PSUM->SBUF fp16, amortizing the per-op fixed cost. Pairs alternate
    between DVE (tensor_copy) and ACT (activation Copy) to split the
    evacuation load; ACT pairs sit late in the sequence, after ACT has
    finished score production.
  - Score pieces are issued just-in-time (a few pair-positions before
    first use) so ACT evacs can interleave with late score pieces
    without blocking them.
  - Junk matmuls at startup warm the PE clock gate; frame indices come
    from a short GpSimd iota extended by cheap DVE adds, keeping the
    DMA wire free for real input/output traffic.
"""

import math
import sys
import types

import numpy as np

# ---------------------------------------------------------------------------
# Optional NTFF-profiling plumbing. The runtime image lacks
# antenv.axon_hooks; wire a stand-in so run_bass_kernel_spmd(trace=True)
# works (used by the dev harness; the plain kernel path never traces).
try:  # pragma: no cover - best effort
    import antenv.axon_hooks  # noqa: F401
except ImportError:
    try:
        _hooks_mod = types.ModuleType("antenv.axon_hooks")
        _hook_box = [None]
        _hooks_mod.set_axon_ntff_profile_hook = (
            lambda hook: _hook_box.__setitem__(0, hook)
        )
        _hooks_mod.get_axon_ntff_profile_hook = lambda: _hook_box[0]
        sys.modules["antenv.axon_hooks"] = _hooks_mod
        from trn_agent_boot.trn_boot import _ntff_profile_via_ctypes

        _hooks_mod.set_axon_ntff_profile_hook(
            _ntff_profile_via_ctypes("/opt/axon/libaxon_pjrt.so")
        )
    except Exception:
        pass

import concourse.bacc as bacc
import concourse.mybir as mybir
import concourse.tile as tile
import concourse.bass_utils as bass_utils
from concourse.tile_rust import add_dep_helper

# Avoid S3 artifact uploads from the trace path in this container.
bass_utils.upload_artifacts = lambda tmpdir: f"local:{tmpdir}"

from concourse.bass_utils import run_bass_kernel_spmd

NCORES = 8
B, S, D, T = 16, 512, 768, 4096
VARIANCE = 10.0
BPC = B // NCORES          # batches per core
P = 128                    # partitions
KC = S // P                # token chunks (4)
MC = T // P                # output frame chunks (32)
QC = MC // 2               # output pair-chunks per slot (16)
DW = D + 1                 # x with the scaled-ones column appended
N0 = 512                   # first matmul column split (one PSUM bank)
XSCALE = 256.0             # keeps fp16 num/den in normal range
MARGIN = 40.0              # frames; exp(-40^2/10) underflows fp32 to 0
ACT_PIECE = 2048           # max free-dim length of one score ACT op
PIECE_LEAD = 4             # issue score pieces this many pairs early

# Evacuation engine per pair-sequence position (32 entries).
#   V: DVE paired tensor_copy   A: ACT paired activation-Copy
# ACT evacs must sit late (ACT is busy producing scores first).
ENG_SEQ = tuple(
    "A" if i in (16, 18, 20, 22, 24, 26, 28, 30, 31) else "V"
    for i in range(32)
)

_PROGRAMS = {}


def _compute_bands(c_masked):
    """Per token-chunk [lo, hi) active frame range (128-aligned), unioned
    over the given batches. c_masked: (n, S) float64, masked tokens nan.
    A fully-masked chunk yields None (skipped entirely)."""
    bands = []
    for k in range(KC):
        ck = c_masked[:, k * P:(k + 1) * P]
        if np.all(np.isnan(ck)):
            bands.append(None)
            continue
        lo = np.nanmin(ck) - MARGIN
        hi = np.nanmax(ck) + MARGIN
        a = max(0, int(math.floor(lo - 1)) // P * P)
        b = min(T, -(-int(math.ceil(hi)) // P) * P)
        b = max(b, a + P)
        bands.append((a, b))
    return tuple(bands)


def _build_program(bands2):
    """bands2: per batch-slot tuple of per-chunk (a, b) bands (or None)."""
    nc = bacc.Bacc("TRN2", target_bir_lowering=False, debug=False)
    f32 = mybir.dt.float32
    fp16 = mybir.dt.float16

    xw_d = nc.dram_tensor("xw", [BPC, S, DW], fp16, kind="ExternalInput").ap()
    bias_d = nc.dram_tensor("bias", [BPC, P, KC], f32,
                        kind="ExternalInput").ap()
    out_d = nc.dram_tensor("out", [BPC, T, DW], fp16,
                           kind="ExternalOutput").ap()

    rsv = 1.0 / math.sqrt(VARIANCE)
    AF = mybir.ActivationFunctionType

    # score pieces (k, t0, t1) in frame order; matmul chunk lists per m
    pieces2, mk2 = [], []
    for bands in bands2:
        pieces = []
        for k, band in enumerate(bands):
            if band is None:
                continue
            a, b = band
            t0 = a
            while t0 < b:
                t1 = min(t0 + ACT_PIECE, b)
                pieces.append((k, t0, t1))
                t0 = t1
        pieces.sort(key=lambda p: (p[1], p[0]))
        if pieces and pieces[0][2] - pieces[0][1] > 1024:
            k, t0, t1 = pieces[0]
            pieces[0:1] = [(k, t0, t0 + 256), (k, t0 + 256, t0 + 512),
                           (k, t0 + 512, t0 + 1024), (k, t0 + 1024, t1)]
        pieces2.append(pieces)
        mk = []
        for m in range(MC):
            ks = [k for k, band in enumerate(bands)
                  if band and m * P < band[1] and (m + 1) * P > band[0]]
            assert ks, f"no active token chunk for m={m}"
            mk.append(ks)
        mk2.append(mk)

    # Pair-sequence: slot 0 leads while slot 1's scores are still being
    # produced, then the two slots interleave.
    pair_seq = [(0, q) for q in range(6)]
    for i in range(10):
        pair_seq.append((0, 6 + i))
        pair_seq.append((1, i))
    pair_seq += [(1, q) for q in range(10, QC)]

    # Just-in-time piece schedule: issue each piece PIECE_LEAD pair
    # positions before the first pair that consumes its chunk's scores.
    first_use = {}
    for seq_idx, (b, q) in enumerate(pair_seq):
        for g in range(2):
            m = 2 * q + g
            for k in mk2[b][m]:
                lo, hi = m * P, (m + 1) * P
                for pi, (pk, t0, t1) in enumerate(pieces2[b]):
                    if pk == k and t0 < hi and t1 > lo:
                        first_use.setdefault((b, pi), seq_idx)
    issue_at = {}
    for (b, pi), use in sorted(first_use.items()):
        issue_at.setdefault(max(0, use - PIECE_LEAD), []).append((b, pi))

    with tile.TileContext(nc) as tc:
        with tc.tile_pool(name="const", bufs=1) as constp, \
             tc.tile_pool(name="sb", bufs=2) as sb, \
             tc.tile_pool(name="outp", bufs=6) as outp, \
             tc.tile_pool(name="colp", bufs=4) as colp, \
             tc.tile_pool(name="ps", bufs=2, space="PSUM") as ps:

            # Warm the ACT table set (exp_and_others) before any real work.
            warm = colp.tile([P, 1], f32, name="warm", tag="warm", bufs=1)
            nc.vector.memset(warm[:], 0.0)
            nc.scalar.activation(warm[:], warm[:], AF.Exp)

            # Warm the PE HAM clock gate: junk matmuls while the real
            # inputs are still loading, so real matmuls run at 2.4GHz.
            junk = constp.tile([P, 512], fp16)
            nc.vector.memset(junk[:], 0.0)
            for _ in range(7):
                jp = ps.tile([P, 512], f32, name="jp", tag="pt")
                nc.tensor.matmul(jp[:], junk[:, 0:P], junk[:],
                                 start=True, stop=True)

            # trow (frame indices 1..T): GpSimd iota for the first 512
            # (feeds the first score pieces early), cheap DVE adds extend.
            trow = constp.tile([P, T], f32)
            nc.gpsimd.iota(trow[:, 0:512],
                           pattern=[[1, 512]], base=1,
                           channel_multiplier=0,
                           allow_small_or_imprecise_dtypes=True)
            for q0 in range(512, T, 512):
                nc.vector.tensor_scalar_add(
                    trow[:, q0:q0 + 512], trow[:, 0:512], float(q0)
                )

            # All input DMAs up front on the Sync queue, before any output
            # issue can block them (the queue drains in program order).
            tiles = []
            for b in range(BPC):
                bcol = colp.tile([P, KC], f32, name="bcol", tag="bcol")
                nc.sync.dma_start(out=bcol[:], in_=bias_d[b])
                xw = sb.tile([P, KC, DW], fp16, name="xw_t", tag="xw_t")
                xw_src = xw_d[b].rearrange("(k p) d -> p k d", p=P)
                for k in range(KC):
                    nc.sync.dma_start(
                        out=xw[:, k:k + 1, :], in_=xw_src[:, k:k + 1, :]
                    )
                tiles.append((bcol, xw))

            score_tiles = [
                sb.tile([P, KC, T], fp16, name="scores", tag="scores")
                for _ in range(BPC)
            ]

            def issue_piece(b, pi):
                bcol, _ = tiles[b]
                k, t0, t1 = pieces2[b][pi]
                u = sb.tile([P, t1 - t0], f32, name="u", tag="u", bufs=5)
                nc.scalar.activation(
                    u[:], trow[:, t0:t1], AF.Square,
                    bias=bcol[:, k:k + 1], scale=rsv,
                )
                nc.scalar.activation(
                    score_tiles[b][:, k, t0:t1], u[:], AF.Exp, scale=-1.0
                )

            for seq_idx, (b, q) in enumerate(pair_seq):
                for pb, pi in issue_at.get(seq_idx, ()):
                    issue_piece(pb, pi)

                bcol, xw = tiles[b]
                scores = score_tiles[b]

                pt2 = ps.tile([P, 2, 1024], f32, name="pt2", tag="pt")
                for g in range(2):
                    m = 2 * q + g
                    ks = mk2[b][m]
                    for i, k in enumerate(ks):
                        lhsT = scores[:, k, m * P:(m + 1) * P]
                        st = (i == 0)
                        sp = (i == len(ks) - 1)
                        mma = nc.tensor.matmul(
                            pt2[:, g, 0:N0], lhsT, xw[:, k, 0:N0],
                            start=st, stop=sp,
                        )
                        mmb = nc.tensor.matmul(
                            pt2[:, g, N0:DW], lhsT, xw[:, k, N0:DW],
                            start=st, stop=sp,
                        )
                        add_dep_helper(mmb.ins, mma.ins,
                                       reason="keep N-pieces adjacent")

                ot = outp.tile([P, 2, DW], fp16, name="ot", tag="ot")
                if ENG_SEQ[seq_idx] == "V":
                    nc.vector.tensor_copy(ot[:], pt2[:, :, 0:DW])
                else:
                    nc.scalar.activation(ot[:], pt2[:, :, 0:DW], AF.Copy)

                nc.sync.dma_start(
                    out=out_d[b, 2 * q * P:(2 * q + 2) * P, :]
                    .rearrange("(g p) d -> p g d", p=P),
                    in_=ot[:],
                )

    nc.compile()
    return nc


def _get_program(bands):
    prog = _PROGRAMS.get(bands)
    if prog is None:
        prog = _build_program(bands)
        _PROGRAMS[bands] = prog
    return prog


def _prepare(x, d, mask):
    x = np.asarray(x, dtype=np.float32)
    d64 = np.asarray(d, dtype=np.float64)
    mask = np.asarray(mask, dtype=bool)

    e = np.cumsum(d64, axis=-1)
    c = e - 0.5 * d64                      # (B, S) token centers
    c_m = np.where(mask, c, np.nan)

    # Sort batches by valid length; slot 0 takes the 8 shortest, slot 1 the
    # 8 longest. Similar lengths per slot give much tighter per-slot bands.
    order = np.argsort(mask.sum(1), kind="stable")
    bands2 = tuple(
        _compute_bands(c_m[order[s * NCORES:(s + 1) * NCORES]])
        for s in range(BPC)
    )

    c = np.where(mask, c, -1.0e4)          # masked tokens: exp underflows to 0
    bias = (-c / math.sqrt(VARIANCE)).astype(np.float32)
    # p-major layout [B, P, KC] so the on-device DMA reads contiguously
    bias = np.ascontiguousarray(bias.reshape(B, KC, P).transpose(0, 2, 1))

    xw = np.empty((B, S, DW), dtype=np.float16)
    xw[:, :, :D] = (x * XSCALE).astype(np.float16)
    xw[:, :, D] = XSCALE

    in_maps = []
    for core in range(NCORES):
        idx = [order[core], order[NCORES + core]]
        in_maps.append({
            "xw": np.ascontiguousarray(xw[idx]),
            "bias": np.ascontiguousarray(bias[idx]),
        })
    return in_maps, bands2, order


def run(x, d, mask, frame_length, trace=False):
    assert int(frame_length) == T
    in_maps, bands2, order = _prepare(x, d, mask)
    nc = _get_program(bands2)
    res = None
    for attempt in range(3):
        try:
            res = run_bass_kernel_spmd(nc, in_maps, list(range(NCORES)),
                                       trace=trace)
            break
        except Exception:
            # The first execution after a fresh compile occasionally hits a
            # transient device error; retrying succeeds.
            if attempt == 2:
                raise
    out = np.empty((B, T, D), dtype=np.float32)
    for core in range(NCORES):
        for s in range(BPC):
            nd = res.results[core]["out"][s].astype(np.float32)
            out[order[s * NCORES + core]] = nd[:, 0:D] / nd[:, D:DW]
    return out, res


def kernel(x, d, mask, frame_length):
    out, _ = run(x, d, mask, frame_length, trace=False)
    return out
